# revision 1
# baseline (speedup 1.0000x reference)
"""BinaryLinear Trainium2 kernel (v4: W-resident fp8 signs, paired chains).

Computes out = x @ (alpha * sign(W)).T + bias where
alpha = mean(|W|, axis=1) (per-output-row scale), for
x [4, 2048, 4096] f32, W [4096, 4096] f32, bias [4096] f32.

Sharding: 2-way over tokens x 4-way over out_features = 8 cores.
Per core: T_c=4096 tokens, O_c=1024 out features, K=4096.

Schedule (per core): sign(W)^T lives resident in SBUF as fp8e4 (+-1 is
exact in fp8; 32KB/partition), built on the fly from a streamed W.
x streams through in 8 blocks of 512 tokens (f32 DMA + bf16 cast on
ScalarE), processed in PAIRS: per (pair, o-block) a 32-deep K-contiguous
PSUM accumulation chain where each Ldweights feeds TWO 512-token
matmuls (halves the stationary-reload overhead; keeps the PE HAM
clock-gate warm at 2.4 GHz). alpha is exact fp32: |W| reduced over k on
DVE, partition-folded by a 1-row fp32 ones-matmul; epilogue is a single
ScalarE op out = psum * alpha + bias fused with the PSUM->SBUF copy.
Loads ride the SP HWDGE ring, stores the Act ring.
"""

import numpy as np

import concourse.bass as bass
import concourse.mybir as mybir
import concourse.tile as tile
from concourse import bacc
from concourse.bass_utils import run_bass_kernel_spmd

F32 = mybir.dt.float32
BF16 = mybir.dt.bfloat16
FP8 = mybir.dt.float8e4

# Full problem shape (hardcoded; kernel.py must be self-contained).
B, S, D_IN, D_OUT = 4, 2048, 4096, 4096
T_FULL = B * S  # 8192 tokens
R_T, C_O = 2, 4  # token-dim shards x out-feature shards = 8 cores
N_CORES = R_T * C_O

P = 128
K = D_IN
KC = K // P              # 32 k-chunks
T_C = T_FULL // R_T      # 4096 tokens per core
O_C = D_OUT // C_O       # 1024 out features per core
NTB = T_C // 512         # 8 t-blocks of 512 tokens
NOB = O_C // P           # 8 o-blocks of 128 out features
NXQ = 8                  # x t-block loaded in 8 sub-chunks of 4 k-chunks
KQ = KC // NXQ           # 4 k-chunks per x sub-chunk
NWH = 2                  # W o-block loaded in 2 halves of 16 k-chunks
KH = KC // NWH


def build_nc(reps=1, ablate=()):
    """Build + compile the per-core Bass program.

    reps: repeat the whole computation (timing only; reps=1 for grading)
    ablate: timing diagnostics; subset of {"loads", "mms", "stores", "veps"}
    """
    nc = bacc.Bacc("TRN2", target_bir_lowering=False, debug=False)

    # host-pretiled inputs: each DMA reads one fully contiguous block
    xT = nc.dram_tensor("xT", [NTB, NXQ, P, KQ, 512], F32, kind="ExternalInput")
    wT = nc.dram_tensor("wT", [NOB, NWH, P, KH, P], F32, kind="ExternalInput")
    biasT = nc.dram_tensor("biasT", [P, NOB], F32, kind="ExternalInput")
    outT = nc.dram_tensor("outT", [NOB, P, T_C], F32, kind="ExternalOutput")

    xT_v = xT.ap()
    wT_v = wT.ap()
    out_v = outT.ap()

    Sign = mybir.ActivationFunctionType.Sign
    Identity = mybir.ActivationFunctionType.Identity

    with tile.TileContext(nc) as tc:
        import contextlib

        with contextlib.ExitStack() as ctx:
            const = ctx.enter_context(tc.tile_pool(name="const", bufs=1))
            st_pool = ctx.enter_context(tc.tile_pool(name="st", bufs=1))
            wstage_pool = ctx.enter_context(tc.tile_pool(name="wstage", bufs=2))
            xstage_pool = ctx.enter_context(tc.tile_pool(name="xstage", bufs=2))
            xbf_pool = ctx.enter_context(tc.tile_pool(name="xbf", bufs=4))
            out_pool = ctx.enter_context(tc.tile_pool(name="out_sb", bufs=4))
            absh_pool = ctx.enter_context(tc.tile_pool(name="absh", bufs=2))
            absk_pool = ctx.enter_context(tc.tile_pool(name="absk", bufs=4))
            psum_pool = ctx.enter_context(
                tc.tile_pool(name="psum", bufs=6, space="PSUM")
            )
            apsum_pool = ctx.enter_context(
                tc.tile_pool(name="apsum", bufs=1, space="PSUM")
            )

            # persistent tensors
            ST = st_pool.tile([P, KC, O_C], FP8, tag="ST")  # sign(W)^T
            biasT_sb = const.tile([P, NOB], F32, tag="biasT")
            alphaT_sb = const.tile([P, NOB], F32, tag="alphaT")
            ones = const.tile([P, 1], F32, tag="ones")

            def body(_=None):
                xbf_tiles = {}
                absk_tiles = {}

                alpha_ps = apsum_pool.tile([P, NOB], F32, tag="aps")

                def x_alloc(t):
                    xbf_tiles[t] = xbf_pool.tile(
                        [P, KC, 512], BF16, tag="xbf", name=f"xbf{t}"
                    )

                def x_chunk(t, q):
                    st = xstage_pool.tile([P, KQ, 512], F32, tag="xs")
                    if "loads" not in ablate:
                        nc.sync.dma_start(st[:], xT_v[t, q])
                    else:
                        nc.vector.memset(st[:, 0, :1], 0.0)
                    nc.scalar.copy(
                        xbf_tiles[t][:, q * KQ : (q + 1) * KQ, :], st[:]
                    )

                def x_load(t):
                    x_alloc(t)
                    for q in range(NXQ):
                        x_chunk(t, q)

                def prep_w(ob):
                    osl = slice(ob * P, (ob + 1) * P)
                    ah = []
                    for h in range(NWH):
                        ws = wstage_pool.tile([P, KH, P], F32, tag="ws")
                        if "loads" not in ablate:
                            nc.sync.dma_start(ws[:], wT_v[ob, h])
                        else:
                            nc.vector.memset(ws[:, 0, :1], 0.0)
                        ksl = slice(h * KH, (h + 1) * KH)
                        nc.scalar.activation(ST[:, ksl, osl], ws[:], Sign)
                        part = absh_pool.tile(
                            [P, P], F32, tag="ah", name=f"ah{ob}_{h}"
                        )
                        nc.vector.tensor_reduce(
                            part[:],
                            ws[:].rearrange("p a b -> p b a"),
                            axis=mybir.AxisListType.X,
                            op=mybir.AluOpType.add,
                            apply_absolute_value=True,
                        )
                        ah.append(part)
                    ab = absk_pool.tile([P, P], F32, tag="ab", name=f"ab{ob}")
                    nc.vector.tensor_add(ab[:], ah[0][:], ah[1][:])
                    absk_tiles[ob] = ab

                def alpha_fin(ob):
                    # fold |W| partials over partitions: exact fp32 1-row matmul
                    nc.tensor.matmul(
                        alpha_ps[:, ob : ob + 1],
                        absk_tiles.pop(ob)[:],
                        ones[:],
                        start=True,
                        stop=True,
                    )
                    nc.vector.tensor_scalar_mul(
                        alphaT_sb[:, ob : ob + 1],
                        alpha_ps[:, ob : ob + 1],
                        1.0 / K,
                    )

                def chain_pair(t0, t1, ob):
                    ptA = psum_pool.tile(
                        [P, 512], F32, tag="pw", name=f"pw_{t0}_{ob}"
                    )
                    ptB = psum_pool.tile(
                        [P, 512], F32, tag="pw", name=f"pw_{t1}_{ob}"
                    )
                    osl = slice(ob * P, (ob + 1) * P)
                    if "mms" not in ablate:
                        for kc in range(KC):
                            st_ap = ST[:, kc, osl]
                            nc.tensor.matmul(
                                ptA[:],
                                st_ap,
                                xbf_tiles[t0][:, kc, :],
                                start=(kc == 0),
                                stop=(kc == KC - 1),
                            )
                            nc.tensor.matmul(
                                ptB[:],
                                st_ap,
                                xbf_tiles[t1][:, kc, :],
                                start=(kc == 0),
                                stop=(kc == KC - 1),
                            )
                    else:
                        nc.vector.memset(ptA[:, :1], 0.0)
                        nc.vector.memset(ptB[:, :1], 0.0)
                    return ptA, ptB

                def epilogue(t, ob, pt):
                    osb = out_pool.tile([P, 512], F32, tag="osb")
                    if "veps" not in ablate:
                        nc.scalar.activation(
                            osb[:],
                            pt[:],
                            Identity,
                            bias=biasT_sb[:, ob : ob + 1],
                            scale=alphaT_sb[:, ob : ob + 1],
                        )
                    else:
                        nc.scalar.copy(osb[:], pt[:])
                    if "stores" not in ablate:
                        nc.scalar.dma_start(
                            out_v[ob][:, t * 512 : (t + 1) * 512], osb[:]
                        )

                # ---- prologue: bias, first W block, first x pair
                nc.sync.dma_start(biasT_sb[:], biasT.ap())
                nc.vector.memset(ones[:], 1.0)
                prep_w(0)
                x_load(0)
                x_load(1)

                # ---- main loop over pairs of t-blocks
                for tp in range(NTB // 2):
                    t0, t1 = 2 * tp, 2 * tp + 1
                    for ob in range(NOB):
                        if tp == 0:
                            alpha_fin(ob)
                            if ob < NOB - 1:
                                prep_w(ob + 1)
                            # interleave next pair's x among W preps
                            if ob == 0:
                                x_alloc(2)
                            if ob == 4:
                                x_alloc(3)
                            x_chunk(2 + ob // 4, (ob % 4) * 2)
                            x_chunk(2 + ob // 4, (ob % 4) * 2 + 1)
                        ptA, ptB = chain_pair(t0, t1, ob)
                        epilogue(t0, ob, ptA)
                        epilogue(t1, ob, ptB)
                    if 1 <= tp < NTB // 2 - 1:
                        x_load(2 * tp + 2)
                        x_load(2 * tp + 3)

            if reps == 1:
                body()
            else:
                with tc.For_i(0, reps, 1) as _i:
                    body()

    nc.compile()
    return nc


_NC_CACHE = {}


def _get_nc(key=1):
    if key not in _NC_CACHE:
        _NC_CACHE[key] = build_nc(reps=key)
    return _NC_CACHE[key]


def pretile_x(x_slice):
    """[T_c, K] f32 -> [NTB, NXQ, P, KQ, 512] (pure permutation)."""
    v = x_slice.reshape(NTB, 512, NXQ, KQ, P)
    return np.ascontiguousarray(v.transpose(0, 2, 4, 3, 1))


def pretile_w(w_slice):
    """[O_c, K] f32 -> [NOB, NWH, P, KH, P] (pure permutation)."""
    u = w_slice.reshape(NOB, P, NWH, KH, P)
    return np.ascontiguousarray(u.transpose(0, 2, 4, 3, 1))


def make_in_maps(x2, w, b):
    xT_shards = [
        pretile_x(x2[i * T_C : (i + 1) * T_C, :]) for i in range(R_T)
    ]
    in_maps = []
    for core in range(N_CORES):
        i, j = core // C_O, core % C_O
        in_maps.append(
            {
                "xT": xT_shards[i],
                "wT": pretile_w(w[j * O_C : (j + 1) * O_C, :]),
                "biasT": np.ascontiguousarray(
                    b[j * O_C : (j + 1) * O_C].reshape(NOB, P).T
                ),
            }
        )
    return in_maps


def kernel(x, weight_real, bias):
    assert x.shape == (B, S, D_IN) and weight_real.shape == (D_OUT, D_IN)
    x2 = np.ascontiguousarray(
        np.asarray(x, dtype=np.float32).reshape(T_FULL, D_IN)
    )
    w = np.asarray(weight_real, dtype=np.float32)
    b = np.asarray(bias, dtype=np.float32)

    in_maps = make_in_maps(x2, w, b)
    nc = _get_nc(1)
    res = run_bass_kernel_spmd(nc, in_maps, core_ids=list(range(N_CORES)))

    out_full = np.empty((T_FULL, D_OUT), dtype=np.float32)
    for core in range(N_CORES):
        i, j = core // C_O, core % C_O
        # outT [NOB, P, T_C] -> [T_C, O_C]
        o = res.results[core]["outT"]
        out_full[i * T_C : (i + 1) * T_C, j * O_C : (j + 1) * O_C] = (
            o.transpose(2, 0, 1).reshape(T_C, O_C)
        )
    return out_full.reshape(B, S, D_OUT)

